# revision 2
# baseline (speedup 1.0000x reference)
"""ConceptBank embedding-lookup kernel for 8 Trainium2 NeuronCores.

Strategy (data-parallel over batch):
  - Each of the 8 cores processes 4 of the 32 sequences; the 1M x 256
    table is replicated per core, pre-rounded to bf16 on host (halves
    gather traffic; mean-of-8182-rows output tolerates the rounding).
  - Rolling polynomial hashes mod (2^61-1) never actually wrap: every
    window hash is a sum of <=5 terms (b+1)*257^k < 2^61-1.  So
    id = hash mod 1e6 = (sum_k (b+1)*(257^k mod 1e6)) mod 1e6, computable
    exactly in fp32 by splitting each constant into hi*4096+lo limbs (all
    intermediates stay < 2^24).
  - ids feed 8 big gpsimd indirect-DMA gathers (4096 rows / call) to
    amortize the ~1us SWDGE fixed cost per call; gathered bf16 blocks are
    tree-reduced on the vector engine (level 1 in bf16 at 2x rate); a
    one-hot PE matmul reduces partitions to the 4 per-sequence sums;
    L2-normalize on-chip.

Layout per core: sequence r owns partitions [32r, 32r+32); position
i = q*64 + c maps to (partition 32r+q, column c); the 4 n-gram sizes
occupy four 64-column blocks of the id space; gather call j covers id
columns [32j, 32j+32).
"""

import sys

sys.path.insert(0, "/opt/trn_rl_repo")

import numpy as np

import concourse.bass as bass
import concourse.mybir as mybir
import concourse.tile as tile

B, T = 32, 2048
V, D = 1000000, 256
NCORES = 8
SEQ_PER_CORE = B // NCORES  # 4
Q = 32  # partitions per sequence
CPART = T // Q  # 64 columns per partition
NN = 4  # n-gram sizes 2..5
KROWS = 32  # id columns per gather call
NCALLS = NN * CPART // KROWS  # 8 gather calls (4096 rows each)
GW = KROWS * D  # 8192 gathered elements per partition per call
CNT_VALID = float(NN * T - (1 + 2 + 3 + 4))  # 8182 valid positions/sequence
RND = 12582912.0  # 1.5*2^23: x+RND-RND rounds fp32 to nearest int

# 257^k mod 1e6, split into hi*4096 + lo
CK = [pow(257, k, 10**6) for k in range(5)]
C_HI = [c >> 12 for c in CK]
C_LO = [c & 4095 for c in CK]
F32, I32, BF16 = mybir.dt.float32, mybir.dt.int32, mybir.dt.bfloat16
COPY = mybir.ActivationFunctionType.Copy
ADD = mybir.AluOpType.add


def _sanitize_waits(nc):
    """This walrus build rejects >1 sync-wait per instruction; move extras
    onto same-engine NOPs inserted just before."""
    for f in nc.m.functions:
        for bb in f.blocks:
            insts = list(bb.instructions)
            out, changed = [], False
            for inst in insts:
                si = inst.sync_info
                waits = list(si.on_wait) if si is not None else []
                if len(waits) > 1:
                    changed = True
                    for w in waits[:-1]:
                        nop = nc.engines[inst.engine].nop(
                            nofuse=True, hint="wsplit"
                        ).ins
                        for bb2 in f.blocks:
                            il = bb2.instructions
                            if il and il[-1].name == nop.name:
                                il.pop()
                                break
                        nop.sync_info = mybir.SyncInfo(on_wait=[w], on_update=[])
                        out.append(nop)
                    si.on_wait = [waits[-1]]
                    inst.sync_info = si
                out.append(inst)
            if changed:
                bb.instructions = out


def _mod_const(nc, pool, w, m, tag):
    """w <- w mod m elementwise, exact for integer-valued fp32 w in
    [0, limit*m) with limit*m < 2^24. Nearest-round quotient leaves
    w - q*m in (-m/2-eps, m/2+eps): one negative fixup suffices."""
    f = pool.tile([128, CPART], F32, tag=f"{tag}_f", name=f"{tag}_f")
    q = pool.tile([128, CPART], F32, tag=f"{tag}_q", name=f"{tag}_q")
    qm = pool.tile([128, CPART], F32, tag=f"{tag}_qm", name=f"{tag}_qm")
    nc.scalar.activation(f[:], w, COPY, scale=1.0 / m)
    nc.vector.tensor_scalar(q[:], f[:], RND, RND, ADD, mybir.AluOpType.subtract)
    nc.scalar.activation(qm[:], q[:], COPY, scale=float(m))
    nc.vector.tensor_tensor(w, w, qm[:], mybir.AluOpType.subtract)
    nc.vector.tensor_scalar(q[:], w, 0.0, float(m), mybir.AluOpType.is_lt,
                            mybir.AluOpType.mult)
    nc.vector.tensor_tensor(w, w, q[:], ADD)


def build_nc(nloop=1):
    """Build the single-core Bass program (SPMD across 8 cores)."""
    nc = bass.Bass("TRN2", target_bir_lowering=False, debug=False, num_devices=1)
    xb = nc.dram_tensor("xb", [SEQ_PER_CORE, T + 4], F32, kind="ExternalInput").ap()
    table = nc.dram_tensor("table", [V, D], BF16, kind="ExternalInput").ap()
    out = nc.dram_tensor("out", [SEQ_PER_CORE, D], F32, kind="ExternalOutput").ap()

    with tile.TileContext(nc) as tc:
        with tc.tile_pool(name="hash", bufs=1) as hp, \
             tc.tile_pool(name="ids", bufs=1) as ip, \
             tc.tile_pool(name="g", bufs=3) as gp, \
             tc.tile_pool(name="tr", bufs=2) as tp, \
             tc.tile_pool(name="part", bufs=2) as accp, \
             tc.tile_pool(name="fin", bufs=1) as fp, \
             tc.tile_pool(name="ps", bufs=1, space="PSUM") as pp:

            # one-hot [128,4] partition->sequence reduction matrix, and the
            # -10 correction row (invalid positions are redirected to id 0)
            onehot = fp.tile([128, NN], F32)
            nc.vector.memset(onehot[:], 0.0)
            for r in range(SEQ_PER_CORE):
                nc.vector.memset(onehot[Q * r:Q * r + Q, r:r + 1], 1.0)
            corr = fp.tile([1, NN], F32)
            nc.vector.memset(corr[:], -10.0)
            t0b = fp.tile([1, D], BF16)
            nc.sync.dma_start(t0b[:], table[0:1, :])
            t0 = fp.tile([1, D], F32)
            nc.vector.tensor_copy(t0[:], t0b[:])

            for rep in range(nloop):
                # ---- byte tile: X[32r+q, c'] = xb_pad[r, q*64 + c'] -------
                # (c' in [0,68); shifted views X[:, 4-k:68-k] give byte
                # streams delayed by k without further DMAs)
                X = hp.tile([128, CPART + 4], F32, tag="X", name="X")
                main = xb[:, 4:4 + T].rearrange("r (q c) -> (r q) c", q=Q)
                head = xb[:, 0:T].rearrange("r (q c) -> (r q) c", q=Q)
                nc.sync.dma_start(X[:, 4:4 + CPART], main)
                nc.sync.dma_start(X[:, 0:4], head[:, 0:4])

                def bk(k):
                    return X[:, 4 - k:4 - k + CPART]

                # ---- hash terms (exact fp32; mul-adds on ACT engine) ------
                plo = [hp.tile([128, CPART], F32, tag=f"plo{k}", name=f"plo{k}")
                       for k in range(5)]
                phi = [None, None] + [
                    hp.tile([128, CPART], F32, tag=f"phi{k}", name=f"phi{k}")
                    for k in range(2, 5)]
                for k in range(5):
                    nc.scalar.activation(plo[k][:], bk(k), COPY,
                                         scale=float(C_LO[k]), bias=float(C_LO[k]))
                    if k >= 2:
                        nc.scalar.activation(phi[k][:], bk(k), COPY,
                                             scale=float(C_HI[k]), bias=float(C_HI[k]))

                ids = [ip.tile([128, KROWS], I32, tag=f"ids{j}", name=f"ids{j}")
                       for j in range(NCALLS)]

                def emit_ids(n_idx, n, seg):
                    # invalid (i < n-1) -> id 0, fixed up by the -10*t0 row
                    for h in range(2):
                        nc.vector.tensor_copy(
                            ids[2 * n_idx + h][:],
                            seg[:, KROWS * h:KROWS * (h + 1)])
                    for r in range(SEQ_PER_CORE):
                        nc.vector.memset(ids[2 * n_idx][Q * r:Q * r + 1, 0:n - 1], 0)

                slo = hp.tile([128, CPART], F32, tag="slo", name="slo")
                # n=2: id = plo0+plo1 < 2^20 directly, no mod needed
                nc.vector.tensor_tensor(slo[:], plo[0][:], plo[1][:], ADD)
                emit_ids(0, 2, slo)
                shi_prev = None
                for n_idx, n in ((1, 3), (2, 4), (3, 5)):
                    k = n - 1
                    nc.vector.tensor_tensor(slo[:], slo[:], plo[k][:], ADD)
                    if n == 3:
                        shi = phi[2]
                    else:
                        shi = hp.tile([128, CPART], F32, tag=f"shi{n}", name=f"shi{n}")
                        nc.vector.tensor_tensor(shi[:], shi_prev[:], phi[k][:], ADD)
                    shi_prev = shi
                    # id = (64*((64*shi) mod 15625) + slo) mod 1e6
                    u = hp.tile([128, CPART], F32, tag=f"u{n}", name=f"u{n}")
                    nc.scalar.activation(u[:], shi[:], COPY, scale=64.0)
                    _mod_const(nc, hp, u[:], 15625, f"m1_{n}")
                    seg = hp.tile([128, CPART], F32, tag=f"seg{n}", name=f"seg{n}")
                    nc.vector.tensor_scalar(seg[:], u[:], 64.0, 0.0,
                                            mybir.AluOpType.mult, ADD)
                    nc.vector.tensor_tensor(seg[:], seg[:], slo[:], ADD)
                    _mod_const(nc, hp, seg[:], 10**6, f"m2_{n}")
                    emit_ids(n_idx, n, seg)

                # ---- gather + tree-reduce ---------------------------------
                part = accp.tile([128, NCALLS * D], F32, tag="part", name="part")
                for j in range(NCALLS):
                    g = gp.tile([128, GW], BF16, tag="g", name=f"g{j}")
                    nc.gpsimd.indirect_dma_start(
                        out=g[:], out_offset=None, in_=table[:],
                        in_offset=bass.IndirectOffsetOnAxis(ap=ids[j][:], axis=0))
                    l1 = tp.tile([128, GW // 2], BF16, tag="l1", name=f"l1_{j}")
                    nc.vector.tensor_tensor(l1[:], g[:, 0:GW // 2],
                                            g[:, GW // 2:GW], ADD)
                    l2 = tp.tile([128, GW // 4], F32, tag="l2", name=f"l2_{j}")
                    nc.vector.tensor_tensor(l2[:], l1[:, 0:GW // 4],
                                            l1[:, GW // 4:GW // 2], ADD)
                    l3 = tp.tile([128, GW // 8], F32, tag="l3", name=f"l3_{j}")
                    nc.vector.tensor_tensor(l3[:], l2[:, 0:GW // 8],
                                            l2[:, GW // 8:GW // 4], ADD)
                    l4 = tp.tile([128, GW // 16], F32, tag="l4", name=f"l4_{j}")
                    nc.vector.tensor_tensor(l4[:], l3[:, 0:GW // 16],
                                            l3[:, GW // 16:GW // 8], ADD)
                    nc.vector.tensor_tensor(part[:, D * j:D * (j + 1)],
                                            l4[:, 0:D], l4[:, D:2 * D], ADD)

                for w in (4, 2, 1):
                    nc.vector.tensor_tensor(part[:, 0:w * D], part[:, 0:w * D],
                                            part[:, w * D:2 * w * D], ADD)

                # ---- reduce partitions -> 4 seqs, correct, normalize ------
                # (the 1/8182 mean scale cancels inside L2-normalize)
                psum = pp.tile([NN, D], F32, tag="psum")
                nc.tensor.matmul(psum[:], lhsT=onehot[:], rhs=part[:, 0:D],
                                 start=True, stop=False)
                nc.tensor.matmul(psum[:], lhsT=corr[:], rhs=t0[:],
                                 start=False, stop=True)
                fin = fp.tile([NN, D], F32, tag="fin")
                nc.vector.tensor_copy(fin[:], psum[:])
                sq = fp.tile([NN, D], F32, tag="sq")
                nc.vector.tensor_tensor(sq[:], fin[:], fin[:], mybir.AluOpType.mult)
                ss = fp.tile([NN, 1], F32, tag="ss")
                nc.vector.tensor_reduce(ss[:], sq[:], axis=mybir.AxisListType.X,
                                        op=ADD)
                nc.vector.tensor_scalar_max(ss[:], ss[:], 1e-24)
                rs = fp.tile([NN, 1], F32, tag="rs")
                nc.scalar.activation(rs[:], ss[:], mybir.ActivationFunctionType.Sqrt)
                nc.vector.reciprocal(rs[:], rs[:])
                nc.vector.tensor_scalar_mul(fin[:], fin[:], rs[:])
                nc.sync.dma_start(out[:], fin[:])

    _sanitize_waits(nc)
    return nc


_CACHED = {}


def _get_nc(nloop=1):
    if nloop not in _CACHED:
        _CACHED[nloop] = build_nc(nloop)
    return _CACHED[nloop]


def make_in_maps(x_bytes, emb_weight):
    import ml_dtypes

    emb = np.asarray(emb_weight, dtype=np.float32).astype(ml_dtypes.bfloat16)
    emb = np.ascontiguousarray(emb)
    xb_all = np.asarray(x_bytes)
    in_maps = []
    for c in range(NCORES):
        sl = xb_all[SEQ_PER_CORE * c:SEQ_PER_CORE * (c + 1)].astype(np.float32)
        xb_pad = np.zeros((SEQ_PER_CORE, T + 4), np.float32)
        xb_pad[:, 4:] = sl
        in_maps.append({"xb": xb_pad, "table": emb})
    return in_maps


def kernel(x_bytes, emb_weight):
    from concourse.bass_utils import run_bass_kernel_spmd

    nc = _get_nc()
    res = run_bass_kernel_spmd(
        nc, make_in_maps(x_bytes, emb_weight), core_ids=list(range(NCORES)))
    return np.concatenate([r["out"] for r in res.results], axis=0)
